# revision 8
# baseline (speedup 1.0000x reference)
"""Trainium2 Bass kernel for CustomConv: 3x3 conv (pad=1, stride=1) + bias + ReLU.

Input  prev_a  [32, 56, 56, 128] f32 (NHWC)
       filter_w [3, 3, 128, 256] f32 (HWIO)
       filter_b [1, 1, 1, 256]   f32
Output [32, 56, 56, 256] f32

Strategy: data-parallel over batch (4 images per core on 8 cores).
Host pre-transposes to NCHW with a 1-px zero-padded ring so each of the
9 filter taps is a strided SBUF view; conv = 9 accumulated matmuls per
output tile (contraction over the 128 input channels on the partition
dim). Matmuls run in fp16 (10 mantissa bits, fp32 PSUM accumulation),
which streams at full PE rate.

Output tiles are processed in groups of (up to) 4 that share the same
128-wide output-channel half, so each tap's weight load is reused by 4
consecutive matmuls (walrus --enable-ldw-opt dedupes the redundant
LDWEIGHTS). This cuts the PE program below the 4-page IRAM boundary,
which removes a trailing instruction-page DMA that otherwise lands at
the very end of the kernel and extends the measured span.

Engine plan: the scalar engine only issues DMAs (weights first, at boot,
~2.5us earlier than the baseline; then the output stores), bias+ReLU
runs as a single fused tensor_scalar on the otherwise-idle vector
engine, the sync ring carries only the x-block stream, and the PE clock
gate is pre-warmed with zero matmuls sized to end just as the first
weights + x block land.
"""
import numpy as np

import concourse.tile as tile
from concourse import bacc, mybir
from concourse import bass_utils

# Disable walrus birsim (compile-time simulation of the kernel) and turn
# on the redundant-LDWEIGHTS elimination: consecutive matmuls that use
# the same stationary operand keep only the first weight load. The NEFF
# is otherwise identical.
_orig_run_command = bass_utils.run_command


def _patched_run_command(argv, **kwargs):
    sub = {"--enable-birsim=true": "--enable-birsim=false"}
    argv = [sub.get(a, a) for a in argv]
    return _orig_run_command(argv, **kwargs)


bass_utils.run_command = _patched_run_command


def _prune_redundant_ldweights(nc):
    """Drop InstLdweights that reload the stationary operand already in the
    PE array. The tile legalizer emits one InstLdweights per InstMatmult;
    with output tiles grouped so 3-4 consecutive matmuls share the same
    weights, the repeats are dead. Runs after tile scheduling (final
    instruction order) and before Bacc.compile (whose
    generate_event_semaphores pass re-legalizes any merged waits).
    Any waits/updates/dependency edges of a dropped LDW move to the
    following matmul, and references to it are remapped there."""
    n_removed = 0
    for block in nc.main_func.blocks:
        insts = block.instructions
        prev_sig = None
        remove = set()
        remap = {}
        pending = None  # LDW just marked for removal; fix up at next PE inst
        for inst in insts:
            if getattr(inst, "engine", None) != mybir.EngineType.PE:
                continue
            if pending is not None:
                # move the dropped LDW's sync + deps onto the next PE inst
                si = pending.sync_info
                if si is not None and (si.on_wait or si.on_update):
                    ti = inst.sync_info
                    if ti is None:
                        inst.sync_info = si
                    else:
                        ti.on_wait.extend(si.on_wait)
                        ti.on_update.extend(si.on_update)
                inst.merge_dependencies_from(pending)
                remap[pending.name] = inst.name
                pending = None
            if isinstance(inst, mybir.InstLdweights):
                sig = str(inst.ins[0])
                if sig == prev_sig:
                    remove.add(inst.name)
                    pending = inst
                    n_removed += 1
                else:
                    prev_sig = sig
            elif not isinstance(inst, mybir.InstMatmult):
                prev_sig = None  # unknown PE inst may clobber weights
        assert pending is None, "block ended on a removed LDW"
        if remove:
            block.instructions = [i for i in insts if i.name not in remove]
            for i in block.instructions:
                i.remap_dependency_names(remap)
            for name in remove:
                nc.inst_map.pop(name, None)
    return n_removed

N_CORES = 8
IMG_PER_CORE = 4
H = 56          # output spatial
HP = 58         # padded input spatial
CIN = 128
COUT = 256
TAPS = [(dy, dx) for dy in range(3) for dx in range(3)]
RG_ROWS = 8     # output rows per tile
NFREE = RG_ROWS * H  # 448 positions per matmul (<= 512 PSUM bank)
# row-groups per image, processed as blocks of 3 / 4 tiles that share a
# weight load: the 3-tile block (out rows 32..55, in rows 32..57) runs
# first so the first x DMA is the smaller one, then the 4-tile block
# (out rows 0..31, in rows 0..33)
BLOCKS = [(4, 3), (0, 4)]

TRACE = False
TRACE_KWARGS = {}
LAST_RESULTS = None
_NC_CACHE = None


def _build():
    nc = bacc.Bacc("TRN2", debug=False, target_bir_lowering=False,
                   num_devices=N_CORES, enable_partition_id=False,
                   monotonic_sem_count=0)
    x_d = nc.dram_tensor("x", [IMG_PER_CORE, CIN, HP, HP],
                         mybir.dt.float16, kind="ExternalInput")
    # j-major weight layout: w[j] is the contiguous [CIN, 9, 128] slab for
    # output channels j*128..j*128+127
    w_d = nc.dram_tensor("w", [2, CIN, 9, 128],
                         mybir.dt.float16, kind="ExternalInput")
    b_d = nc.dram_tensor("b", [CIN, 2], mybir.dt.float32, kind="ExternalInput")
    o_d = nc.dram_tensor("o", [IMG_PER_CORE, 2, 128, H * H],
                         mybir.dt.float32, kind="ExternalOutput")

    with tile.TileContext(nc) as tc:
        with (tc.tile_pool(name="wb", bufs=10) as wbp,
              tc.tile_pool(name="x", bufs=4) as xp,
              tc.tile_pool(name="o", bufs=8) as op,
              tc.tile_pool(name="ps", bufs=8, space="PSUM") as pp):
            # weight halves split across the two DGE rings so the first
            # half (all the j=0 groups need) lands ~1us earlier; with no
            # scalar activations in the kernel the scalar engine has no
            # ACT_TABLE_LOAD, so its descriptors go out right at boot
            wts = [wbp.tile([CIN, 9, 128], mybir.dt.float16, tag=f"w{j}",
                            name=f"w{j}") for j in range(2)]
            nc.sync.dma_start(wts[0][:], w_d.ap()[0])
            nc.scalar.dma_start(wts[1][:], w_d.ap()[1])
            bt = wbp.tile([CIN, 2], mybir.dt.float32, tag="bias")
            nc.scalar.dma_start(bt[:], b_d.ap())

            # pre-warm the PE clock gate (HAM) with zero matmuls while the
            # weight/input DMAs are in flight, so real matmuls start at the
            # full 2.4 GHz instead of the cold 1.2 GHz
            warm = wbp.tile([CIN, NFREE], mybir.dt.float16, tag="warm")
            nc.gpsimd.memset(warm[:], 0.0)

            # fixed rotating tile sets keep the Tile release/semaphore
            # machinery small
            xts = [xp.tile([CIN, 34, HP], mybir.dt.float16,
                           tag="xblk", name=f"xblk{k}") for k in range(4)]
            ots = [op.tile([128, NFREE], mybir.dt.float32,
                           tag="og", name=f"og{k}") for k in range(8)]
            pss = [pp.tile([128, NFREE], mybir.dt.float32,
                           tag="psg", name=f"psg{k}") for k in range(8)]

            nblk = len(BLOCKS)

            def x_dma(img, blk):
                rg0, ntile = BLOCKS[blk]
                rows = ntile * RG_ROWS + 2
                xt = xts[(img * nblk + blk) % 4]
                # the very first x block rides the scalar ring (issued
                # before the w1 descriptor), in parallel with w0 on the
                # sync ring; the rest of the stream lives on sync
                eng = nc.scalar if (img == 0 and blk == 0) else nc.sync
                eng.dma_start(xt[:, 0:rows, :],
                              x_d.ap()[img, :, rg0 * RG_ROWS:
                                       rg0 * RG_ROWS + rows, :])
                return xt

            first_xt = x_dma(0, 0)

            wps = pss[7]
            for i in range(14):
                n = NFREE if i < 12 else 112
                nc.tensor.matmul(wps[:, 0:n], warm[:, 0:128], warm[:, 0:n],
                                 start=True, stop=True)

            g = 0
            for img in range(IMG_PER_CORE):
                for blk in range(nblk):
                    rg0, ntile = BLOCKS[blk]
                    xt = first_xt if (img == 0 and blk == 0) \
                        else x_dma(img, blk)
                    last = img == IMG_PER_CORE - 1 and blk == nblk - 1
                    for j in range(2):
                        pse = [pss[(g + i) % 8] for i in range(ntile)]
                        if last and j == 1:
                            # final group runs tile-major so only the last
                            # tile's bias+relu + store trail the last
                            # matmul; the other tiles drain during the
                            # preceding ones' matmuls
                            for i in range(ntile):
                                for t, (dy, dx) in enumerate(TAPS):
                                    nc.tensor.matmul(
                                        pse[i][:],
                                        wts[j][:, t, :],
                                        xt[:, dy + i * RG_ROWS:
                                           dy + i * RG_ROWS + RG_ROWS,
                                           dx: dx + H],
                                        start=(t == 0), stop=(t == 8),
                                    )
                                ot = ots[(g + i) % 8]
                                rg = rg0 + i
                                nc.vector.tensor_scalar(
                                    ot[:], pse[i][:], bt[:, j:j + 1], 0.0,
                                    mybir.AluOpType.add, mybir.AluOpType.max)
                                nc.scalar.dma_start(
                                    o_d.ap()[img, j, :,
                                             rg * NFREE:(rg + 1) * NFREE],
                                    ot[:])
                            g += ntile
                            continue
                        for t, (dy, dx) in enumerate(TAPS):
                            for i in range(ntile):
                                nc.tensor.matmul(
                                    pse[i][:],
                                    wts[j][:, t, :],
                                    xt[:, dy + i * RG_ROWS:
                                       dy + i * RG_ROWS + RG_ROWS,
                                       dx: dx + H],
                                    start=(t == 0), stop=(t == 8),
                                )
                        for i in range(ntile):
                            ot = ots[g % 8]
                            ps = pse[i]
                            g += 1
                            rg = rg0 + i
                            nc.vector.tensor_scalar(
                                ot[:], ps[:], bt[:, j:j + 1], 0.0,
                                mybir.AluOpType.add, mybir.AluOpType.max)
                            nc.scalar.dma_start(
                                o_d.ap()[img, j, :,
                                         rg * NFREE:(rg + 1) * NFREE],
                                ot[:])
    _prune_redundant_ldweights(nc)
    nc.compile()
    return nc


def kernel(prev_a, filter_w, filter_b):
    global LAST_RESULTS, _NC_CACHE
    from concourse.bass_utils import run_bass_kernel_spmd

    prev_a = np.asarray(prev_a, dtype=np.float32)
    filter_w = np.asarray(filter_w, dtype=np.float32)
    filter_b = np.asarray(filter_b, dtype=np.float32)

    n = prev_a.shape[0]
    xpad = np.zeros((n, CIN, HP, HP), dtype=np.float16)
    xpad[:, :, 1:1 + H, 1:1 + H] = prev_a.transpose(0, 3, 1, 2).astype(np.float16)
    # [2, CIN, 9, 128]: j-major so each output-channel half is contiguous
    w = np.ascontiguousarray(
        filter_w.reshape(9, CIN, 2, 128).transpose(2, 1, 0, 3).astype(np.float16))
    b = np.ascontiguousarray(filter_b.reshape(2, 128).T)

    if _NC_CACHE is None:
        _NC_CACHE = _build()
    nc = _NC_CACHE

    in_maps = [
        {"x": np.ascontiguousarray(xpad[c * IMG_PER_CORE:(c + 1) * IMG_PER_CORE]),
         "w": w, "b": b}
        for c in range(N_CORES)
    ]
    LAST_RESULTS = run_bass_kernel_spmd(
        nc, in_maps, core_ids=list(range(N_CORES)), trace=TRACE,
        **TRACE_KWARGS)

    outs = []
    for c in range(N_CORES):
        o = LAST_RESULTS.results[c]["o"]  # [4, 2, 128, 3136]
        outs.append(o.reshape(IMG_PER_CORE, COUT, H, H).transpose(0, 2, 3, 1))
    return np.ascontiguousarray(np.concatenate(outs, axis=0))


# revision 10
# speedup vs baseline: 1.0742x; 1.0742x over previous
"""Trainium2 Bass kernel for CustomConv: 3x3 conv (pad=1, stride=1) + bias + ReLU.

Input  prev_a  [32, 56, 56, 128] f32 (NHWC)
       filter_w [3, 3, 128, 256] f32 (HWIO)
       filter_b [1, 1, 1, 256]   f32
Output [32, 56, 56, 256] f32

Strategy: data-parallel over batch (4 images per core on 8 cores).
Host pre-transposes to NCHW with a 1-px zero-padded ring so each of the
9 filter taps is a strided SBUF view; conv = 9 accumulated matmuls per
output tile (contraction over the 128 input channels on the partition
dim). Matmuls run in fp16 (10 mantissa bits, fp32 PSUM accumulation),
which streams at full PE rate.

Output tiles are processed in groups of (up to) 4 that share the same
128-wide output-channel half, so each tap's weight load is reused by 4
consecutive matmuls (walrus --enable-ldw-opt dedupes the redundant
LDWEIGHTS). This cuts the PE program below the 4-page IRAM boundary,
which removes a trailing instruction-page DMA that otherwise lands at
the very end of the kernel and extends the measured span.

Engine plan: the scalar engine only issues DMAs (weights first, at boot,
~2.5us earlier than the baseline; then the output stores), bias+ReLU
runs as a single fused tensor_scalar on the otherwise-idle vector
engine, the sync ring carries only the x-block stream, and the PE clock
gate is pre-warmed with zero matmuls sized to end just as the first
weights + x block land.
"""
import numpy as np

import concourse.tile as tile
from concourse import bacc, mybir
from concourse import bass_utils

# Disable walrus birsim (compile-time simulation of the kernel) and turn
# on the redundant-LDWEIGHTS elimination: consecutive matmuls that use
# the same stationary operand keep only the first weight load. The NEFF
# is otherwise identical.
_orig_run_command = bass_utils.run_command


def _patched_run_command(argv, **kwargs):
    sub = {"--enable-birsim=true": "--enable-birsim=false"}
    argv = [sub.get(a, a) for a in argv]
    return _orig_run_command(argv, **kwargs)


bass_utils.run_command = _patched_run_command


def _prune_redundant_ldweights(nc):
    """Drop InstLdweights that reload the stationary operand already in the
    PE array. The tile legalizer emits one InstLdweights per InstMatmult;
    with output tiles grouped so 3-4 consecutive matmuls share the same
    weights, the repeats are dead. Runs after tile scheduling (final
    instruction order) and before Bacc.compile (whose
    generate_event_semaphores pass re-legalizes any merged waits).
    Any waits/updates/dependency edges of a dropped LDW move to the
    following matmul, and references to it are remapped there."""
    n_removed = 0
    for block in nc.main_func.blocks:
        insts = block.instructions
        prev_sig = None
        remove = set()
        remap = {}
        pending = None  # LDW just marked for removal; fix up at next PE inst
        for inst in insts:
            if getattr(inst, "engine", None) != mybir.EngineType.PE:
                continue
            if pending is not None:
                # move the dropped LDW's sync + deps onto the next PE inst
                si = pending.sync_info
                if si is not None and (si.on_wait or si.on_update):
                    ti = inst.sync_info
                    if ti is None:
                        inst.sync_info = si
                    else:
                        ti.on_wait.extend(si.on_wait)
                        ti.on_update.extend(si.on_update)
                inst.merge_dependencies_from(pending)
                remap[pending.name] = inst.name
                pending = None
            if isinstance(inst, mybir.InstLdweights):
                sig = str(inst.ins[0])
                if sig == prev_sig:
                    remove.add(inst.name)
                    pending = inst
                    n_removed += 1
                else:
                    prev_sig = sig
            elif not isinstance(inst, mybir.InstMatmult):
                prev_sig = None  # unknown PE inst may clobber weights
        assert pending is None, "block ended on a removed LDW"
        if remove:
            block.instructions = [i for i in insts if i.name not in remove]
            for i in block.instructions:
                i.remap_dependency_names(remap)
            for name in remove:
                nc.inst_map.pop(name, None)
    return n_removed

N_CORES = 8
IMG_PER_CORE = 4
H = 56          # output spatial
HP = 58         # padded input spatial
CIN = 128
COUT = 256
TAPS = [(dy, dx) for dy in range(3) for dx in range(3)]
RG_ROWS = 8     # output rows per tile
NFREE = RG_ROWS * H  # 448 positions per matmul (<= 512 PSUM bank)
# row-groups per image, processed as blocks of 3 / 4 tiles that share a
# weight load: the 3-tile block (out rows 32..55, in rows 32..57) runs
# first so the first x DMA is the smaller one, then the 4-tile block
# (out rows 0..31, in rows 0..33)
BLOCKS = [(4, 3), (0, 4)]

TRACE = False
TRACE_KWARGS = {}
LAST_RESULTS = None
_NC_CACHE = None


def _build():
    nc = bacc.Bacc("TRN2", debug=False, target_bir_lowering=False,
                   num_devices=N_CORES, enable_partition_id=False,
                   monotonic_sem_count=0)
    x_d = nc.dram_tensor("x", [IMG_PER_CORE, CIN, HP, HP],
                         mybir.dt.float16, kind="ExternalInput")
    # j-major weight layout: w[j] is the contiguous [CIN, 9, 128] slab for
    # output channels j*128..j*128+127
    w_d = nc.dram_tensor("w", [2, CIN, 9, 128],
                         mybir.dt.float16, kind="ExternalInput")
    b_d = nc.dram_tensor("b", [CIN, 2], mybir.dt.float32, kind="ExternalInput")
    o_d = nc.dram_tensor("o", [IMG_PER_CORE, 2, 128, H * H],
                         mybir.dt.float32, kind="ExternalOutput")

    with tile.TileContext(nc) as tc:
        with (tc.tile_pool(name="wb", bufs=10) as wbp,
              tc.tile_pool(name="x", bufs=4) as xp,
              tc.tile_pool(name="o", bufs=8) as op,
              tc.tile_pool(name="ps", bufs=8, space="PSUM") as pp):
            # weight halves split across the two DGE rings: w0 (all the
            # j=0 groups need) goes first on the sync ring, immediately
            # followed by the first x block; w1/bias ride the scalar ring
            # in parallel (no ACT_TABLE_LOAD there since the kernel has no
            # scalar activations)
            wts = [wbp.tile([CIN, 9, 128], mybir.dt.float16, tag=f"w{j}",
                            name=f"w{j}") for j in range(2)]
            nc.sync.dma_start(wts[0][:], w_d.ap()[0])
            nc.scalar.dma_start(wts[1][:], w_d.ap()[1])
            bt = wbp.tile([CIN, 2], mybir.dt.float32, tag="bias")
            nc.scalar.dma_start(bt[:], b_d.ap())

            # pre-warm the PE clock gate (HAM) with zero matmuls while the
            # weight/input DMAs are in flight, so real matmuls start at the
            # full 2.4 GHz instead of the cold 1.2 GHz
            warm = wbp.tile([CIN, NFREE], mybir.dt.float16, tag="warm")
            nc.gpsimd.memset(warm[:], 0.0)

            # fixed rotating tile sets keep the Tile release/semaphore
            # machinery small
            xts = [xp.tile([CIN, 34, HP], mybir.dt.float16,
                           tag="xblk", name=f"xblk{k}") for k in range(4)]
            ots = [op.tile([128, NFREE], mybir.dt.float32,
                           tag="og", name=f"og{k}") for k in range(8)]
            pss = [pp.tile([128, NFREE], mybir.dt.float32,
                           tag="psg", name=f"psg{k}") for k in range(8)]

            nblk = len(BLOCKS)

            def x_dma(img, blk):
                rg0, ntile = BLOCKS[blk]
                rows = ntile * RG_ROWS + 2
                xt = xts[(img * nblk + blk) % 4]
                r_d = x_d.ap()[img, :, rg0 * RG_ROWS:rg0 * RG_ROWS + rows, :]
                if img == 0 and blk == 0:
                    # first block in two chunks right behind w0 on the sync
                    # ring, so the first tiles' matmuls can start while the
                    # rest of the block is still in flight
                    nc.sync.dma_start(xt[:, 0:18, :], r_d[:, 0:18, :])
                    nc.sync.dma_start(xt[:, 18:rows, :], r_d[:, 18:rows, :])
                else:
                    nc.sync.dma_start(xt[:, 0:rows, :], r_d)
                return xt

            first_xt = x_dma(0, 0)

            wps = pss[7]
            for i in range(15):
                n = NFREE if i < 13 else 112
                nc.tensor.matmul(wps[:, 0:n], warm[:, 0:128], warm[:, 0:n],
                                 start=True, stop=True)

            g = 0
            for img in range(IMG_PER_CORE):
                for blk in range(nblk):
                    rg0, ntile = BLOCKS[blk]
                    xt = first_xt if (img == 0 and blk == 0) \
                        else x_dma(img, blk)
                    last = img == IMG_PER_CORE - 1 and blk == nblk - 1
                    for j in range(2):
                        pse = [pss[(g + i) % 8] for i in range(ntile)]
                        if last and j == 1:
                            # final group runs tile-major so only the last
                            # tile's bias+relu + store trail the last
                            # matmul; the other tiles drain during the
                            # preceding ones' matmuls
                            for i in range(ntile):
                                for t, (dy, dx) in enumerate(TAPS):
                                    nc.tensor.matmul(
                                        pse[i][:],
                                        wts[j][:, t, :],
                                        xt[:, dy + i * RG_ROWS:
                                           dy + i * RG_ROWS + RG_ROWS,
                                           dx: dx + H],
                                        start=(t == 0), stop=(t == 8),
                                    )
                                ot = ots[(g + i) % 8]
                                rg = rg0 + i
                                nc.vector.tensor_scalar(
                                    ot[:], pse[i][:], bt[:, j:j + 1], 0.0,
                                    mybir.AluOpType.add, mybir.AluOpType.max)
                                nc.scalar.dma_start(
                                    o_d.ap()[img, j, :,
                                             rg * NFREE:(rg + 1) * NFREE],
                                    ot[:])
                            g += ntile
                            continue
                        for t, (dy, dx) in enumerate(TAPS):
                            for i in range(ntile):
                                nc.tensor.matmul(
                                    pse[i][:],
                                    wts[j][:, t, :],
                                    xt[:, dy + i * RG_ROWS:
                                       dy + i * RG_ROWS + RG_ROWS,
                                       dx: dx + H],
                                    start=(t == 0), stop=(t == 8),
                                )
                        for i in range(ntile):
                            ot = ots[g % 8]
                            ps = pse[i]
                            g += 1
                            rg = rg0 + i
                            nc.vector.tensor_scalar(
                                ot[:], ps[:], bt[:, j:j + 1], 0.0,
                                mybir.AluOpType.add, mybir.AluOpType.max)
                            nc.scalar.dma_start(
                                o_d.ap()[img, j, :,
                                         rg * NFREE:(rg + 1) * NFREE],
                                ot[:])
    _prune_redundant_ldweights(nc)
    nc.compile()
    return nc


def kernel(prev_a, filter_w, filter_b):
    global LAST_RESULTS, _NC_CACHE
    from concourse.bass_utils import run_bass_kernel_spmd

    prev_a = np.asarray(prev_a, dtype=np.float32)
    filter_w = np.asarray(filter_w, dtype=np.float32)
    filter_b = np.asarray(filter_b, dtype=np.float32)

    n = prev_a.shape[0]
    xpad = np.zeros((n, CIN, HP, HP), dtype=np.float16)
    xpad[:, :, 1:1 + H, 1:1 + H] = prev_a.transpose(0, 3, 1, 2).astype(np.float16)
    # [2, CIN, 9, 128]: j-major so each output-channel half is contiguous
    w = np.ascontiguousarray(
        filter_w.reshape(9, CIN, 2, 128).transpose(2, 1, 0, 3).astype(np.float16))
    b = np.ascontiguousarray(filter_b.reshape(2, 128).T)

    if _NC_CACHE is None:
        _NC_CACHE = _build()
    nc = _NC_CACHE

    in_maps = [
        {"x": np.ascontiguousarray(xpad[c * IMG_PER_CORE:(c + 1) * IMG_PER_CORE]),
         "w": w, "b": b}
        for c in range(N_CORES)
    ]
    LAST_RESULTS = run_bass_kernel_spmd(
        nc, in_maps, core_ids=list(range(N_CORES)), trace=TRACE,
        **TRACE_KWARGS)

    outs = []
    for c in range(N_CORES):
        o = LAST_RESULTS.results[c]["o"]  # [4, 2, 128, 3136]
        outs.append(o.reshape(IMG_PER_CORE, COUT, H, H).transpose(0, 2, 3, 1))
    return np.ascontiguousarray(np.concatenate(outs, axis=0))
